# revision 1
# baseline (speedup 1.0000x reference)
"""Trainium2 Bass kernel for nn_DiscreteSequenceModel (GRU rollout) — v2.

Math (see reference): h0 = y0 @ enc_w.T + enc_b, then T=512 sequential GRU
steps with input == hidden (ts values are unused by the math; only len(ts)
matters), emitting pred_t = h_t @ dec_w.T + dec_b before each update.

Sharding: data-parallel over batch B=256 across 8 cores (32 rows/core),
weights replicated.  Per core, each step contracts h (K=1024, 8 k-tiles of
128) against a packed weight matrix with 4352 output columns
(r|z|hn|inn|pred per feature-group).  To keep the 128x128 PE array full
with only 32 batch rows, we column-tile the array into 4 groups of 32:
every group holds the same stationary h-tile but streams a different
feature-chunk of the weights, so the array does 4 concurrent 32-wide
matmuls (128 effective rows).

v2 scheduling notes vs v1:
 - PSUM double-buffered by sub-step parity: the next step's matmul passes
   never wait on the previous step's PSUM readers.
 - One sigmoid over [r|z] (Act fixed cost is 352 cycles/inst); 1-z and
   z*h moved to the otherwise-idle Pool (GpSimd) engine (SBUF-only ops).
 - The inn pass is split L/R so tanh(left) starts while the PE still
   streams inn_R|pred; the h' left half lands just in time for the next
   step's transpose, keeping the PE stall-free.
"""

import numpy as np

import concourse.bacc as bacc
import concourse.bass as bass
import concourse.tile as tile
from concourse import mybir
from concourse.bass_utils import run_bass_kernel_spmd

B, T, H, D = 256, 512, 1024, 256
NCORES = 8
BL = B // NCORES            # 32 batch rows per core
NG = 4                      # PE column-tile groups
FC = H // NG                # 256 gate features per group
PC = D // NG                # 64 decoder features per group
GW = 4 * FC + PC            # 1088 packed weight cols per group
KT = H // 128               # 8 k-tiles

# free-dim offsets inside one group's 1088-wide strip: [r|z|hn|inn|pred]
RZ0, RZ1 = 0, 2 * FC             # [r 256 | z 256]
HN0, HN1 = 2 * FC, 3 * FC        # [hn 256]
IA0, IA1 = 3 * FC, 3 * FC + 128  # [inn_L 128]
IB0, IB1 = 3 * FC + 128, GW      # [inn_R 128 | pred 64]

# blob column layout (one [128, BLOBW] fp16 tensor holds every constant)
OFF_WS = 0                       # weights: [p, k*4352 + j*1088 + c]
OFF_ES = OFF_WS + KT * NG * GW   # encoder weights: [p, k*1024 + j*256 + c]
OFF_Y0T = OFF_ES + 2 * H         # y0^T k-tiles: [p, k*32 + b]
OFF_BR = OFF_Y0T + 2 * BL        # row 0: packed gate/pred bias row (4352)
OFF_EB = OFF_BR + NG * GW        # row 0: packed encoder bias row (1024)
OFF_ID = OFF_EB + H              # 128x128 identity (for PE transpose)
OFF_ONES = OFF_ID + 128          # row 0: 32 ones (bias k-tile stationary)
BLOBW = OFF_ONES + BL

F32 = mybir.dt.float32
FP16 = mybir.dt.float16
AFT = mybir.ActivationFunctionType
ALU = mybir.AluOpType


def _emit(tc, nc, blob, idf, preds, steps, unroll, clobber=False,
          variant="full"):
    assert steps % unroll == 0 and unroll % 2 == 0
    import contextlib

    do_mm = variant in ("full", "mmonly")
    do_tail = variant in ("full", "tailonly")

    with contextlib.ExitStack() as ctx:
        const = ctx.enter_context(tc.tile_pool(name="const", bufs=1))
        C = const.tile([128, BLOBW], FP16)
        # h state as separate L/R half-tiles so the elementwise tail can
        # release each half to the PE transposes independently
        HL = [const.tile([128, 128], F32, name="HL0"),
              const.tile([128, 128], F32, name="HL1")]
        HR = [const.tile([128, 128], F32, name="HR0"),
              const.tile([128, 128], F32, name="HR1")]

        IDT = const.tile([128, 128], F32)
        nc.sync.dma_start(C[:], blob[:])
        nc.sync.dma_start(IDT[:], idf[:])

        def ws(k, j, c0, c1):
            o = OFF_WS + k * NG * GW + j * GW
            return C[:, o + c0:o + c1]

        ID = IDT[:]
        ONES = C[0:1, OFF_ONES:OFF_ONES + BL]

        ps = ctx.enter_context(tc.tile_pool(name="ps", bufs=1, space="PSUM"))
        # parity-double-buffered matmul accumulators
        ps_rz = [ps.tile([128, 2 * FC], F32, name="ps_rz0"),
                 ps.tile([128, 2 * FC], F32, name="ps_rz1")]
        ps_hn = [ps.tile([128, FC], F32, name="ps_hn0"),
                 ps.tile([128, FC], F32, name="ps_hn1")]
        ps_pi = [ps.tile([128, FC + PC], F32, name="ps_pi0"),
                 ps.tile([128, FC + PC], F32, name="ps_pi1")]
        # transpose scratch: single-buffered (the SBUF copy consumes it
        # immediately; each tile burns a whole 2KB PSUM bank and only 8 exist)
        ps_t1_s = ps.tile([128, 128], F32, name="ps_t1")
        ps_t2_s = ps.tile([128, 128], F32, name="ps_t2")

        sb = ctx.enter_context(tc.tile_pool(name="sb", bufs=2))

        # ---- encoder: h0 in gate layout [32j+b, f_local] ----
        for j in range(NG):
            for k in range(2):
                nc.tensor.matmul(
                    ps_rz[0][32 * j:32 * j + 32, 0:FC],
                    C[:, OFF_Y0T + k * BL:OFF_Y0T + (k + 1) * BL],
                    C[:, OFF_ES + k * H + j * FC:
                      OFF_ES + k * H + (j + 1) * FC],
                    start=(k == 0), stop=False, skip_group_check=True,
                    tile_position=(0, 32 * j))
            nc.tensor.matmul(
                ps_rz[0][32 * j:32 * j + 32, 0:FC],
                ONES,
                C[0:1, OFF_EB + j * FC:OFF_EB + (j + 1) * FC],
                start=False, stop=True, skip_group_check=True,
                tile_position=(0, 32 * j))
        nc.scalar.copy(HL[0][:], ps_rz[0][:, 0:128])
        nc.vector.tensor_copy(HR[0][:], ps_rz[0][:, 128:256])

        if variant == "mmonly":  # static stationary tiles, no per-step dep
            HTE = const.tile([128, 128], FP16, name="HTE")
            HTO = const.tile([128, 128], FP16, name="HTO")
            nc.scalar.copy(HTE[:], C[:, 0:128])
            nc.vector.tensor_copy(HTO[:], C[:, 0:128])
        if variant == "tailonly":  # PSUM read w/o writer needs an alloc+init
            for p in range(2):
                nc.vector.memset(ps_rz[p][:], 0.0)
                nc.vector.memset(ps_hn[p][:], 0.0)
                nc.vector.memset(ps_pi[p][:], 0.0)

        def step_body(tv, sub, stage):
            parity = sub % 2
            hcl, hcr = HL[parity][:], HR[parity][:]
            hnl, hnr = HL[1 - parity][:], HR[1 - parity][:]
            prz, phn, ppi = ps_rz[parity], ps_hn[parity], ps_pi[parity]
            pt1, pt2 = ps_t1_s, ps_t2_s

            if variant == "mmonly":
                hTe, hTo = HTE, HTO
            else:
                hTe = sb.tile([128, 128], FP16, tag="hTe")
                hTo = sb.tile([128, 128], FP16, tag="hTo")

            def mm(out_ap, k, j, c0, c1, start, stop):
                if k < KT:
                    src = hTe if k % 2 == 0 else hTo
                    m = k // 2
                    lhsT = src[:, m * 32:(m + 1) * 32]
                    rhs = ws(k, j, c0, c1)
                else:  # bias "k-tile": ones x bias row
                    lhsT = ONES
                    rhs = C[0:1, OFF_BR + j * GW + c0:OFF_BR + j * GW + c1]
                nc.tensor.matmul(out_ap, lhsT, rhs,
                                 start=start, stop=stop,
                                 skip_group_check=True,
                                 tile_position=(0, 32 * j))

            # bias "k-tiles" first: constants only, so the PE runs them
            # while the previous sub-step's elementwise tail finishes.
            if do_mm:
                for j in range(NG):
                    mm(prz[32 * j:32 * j + 32, :], KT, j, RZ0, RZ1,
                       True, False)
                for j in range(NG):
                    mm(phn[32 * j:32 * j + 32, :], KT, j, HN0, HN1,
                       True, False)
                # single start=True bias mm covering the whole ps_pi tile:
                # one accumulation group per PSUM bank (the first_mm bank
                # clear makes column-split groups unsafe); the k-tile
                # STREAMS below still split L/R for earlier tanh.
                for j in range(NG):
                    mm(ppi[32 * j:32 * j + 32, 0:320], KT, j, IA0, IB1,
                       True, False)

            # left-half transpose as soon as h_left lands, then the even
            # k-tiles (which live in hTe) while the right half finishes
            if do_tail:
                nc.tensor.transpose(pt1[:], hcl, ID)
                nc.vector.tensor_copy(hTe[:], pt1[:])
            if do_mm:
                for k in range(0, KT, 2):
                    for j in range(NG):
                        mm(prz[32 * j:32 * j + 32, :], k, j, RZ0, RZ1,
                           False, False)
            if do_tail:
                nc.tensor.transpose(pt2[:], hcr, ID)
                nc.vector.tensor_copy(hTo[:], pt2[:])
            if do_mm:
                for k in range(1, KT, 2):
                    for j in range(NG):
                        mm(prz[32 * j:32 * j + 32, :], k, j, RZ0, RZ1,
                           False, k == KT - 1)

            if do_tail:
                rzs = sb.tile([128, 2 * FC], F32, tag="rzs")
                omz = sb.tile([128, FC], F32, tag="omz")
                zh = sb.tile([128, FC], F32, tag="zh")
                # r|z in one Act op (amortize the 352-cycle fixed cost)
                nc.scalar.activation(rzs[:], prz[:], AFT.Sigmoid)
                r_, z_ = rzs[:, 0:FC], rzs[:, FC:2 * FC]
                # Pool (SBUF-only engine): 1-z and z*h off the Vector queue
                nc.gpsimd.tensor_scalar(omz[:], z_, -1.0, 1.0,
                                        ALU.mult, ALU.add)
                nc.gpsimd.tensor_mul(zh[:, 0:128], rzs[:, FC:FC + 128], hcl)
                nc.gpsimd.tensor_mul(zh[:, 128:256],
                                     rzs[:, FC + 128:2 * FC], hcr)

            # pass B1: hn — finishes early so v = r*hn hides under B2
            if do_mm:
                for k in range(KT):
                    for j in range(NG):
                        mm(phn[32 * j:32 * j + 32, :], k, j, HN0, HN1,
                           False, k == KT - 1)

            if do_tail:
                v = sb.tile([128, FC], F32, tag="v")
                nc.vector.tensor_mul(v[:], r_, phn[:])

            # pass B2a: inn_L
            if do_mm:
                for k in range(KT):
                    for j in range(NG):
                        mm(ppi[32 * j:32 * j + 32, 0:128], k, j, IA0, IA1,
                           False, k == KT - 1)

            if do_tail:
                w2l = sb.tile([128, 128], F32, tag="w2l")
                ntl = sb.tile([128, 128], F32, tag="ntl")
                nc.vector.tensor_add(w2l[:], v[:, 0:128], ppi[:, 0:128])
                nc.scalar.activation(ntl[:], w2l[:], AFT.Tanh)

            # pass B2b: inn_R | pred
            if do_mm:
                for k in range(KT):
                    for j in range(NG):
                        mm(ppi[32 * j:32 * j + 32, 128:320], k, j, IB0, IB1,
                           False, k == KT - 1)

            if not do_tail:
                return

            w2r = sb.tile([128, 128], F32, tag="w2r")
            ntr = sb.tile([128, 128], F32, tag="ntr")
            t4l = sb.tile([128, 128], F32, tag="t4l")
            t4r = sb.tile([128, 128], F32, tag="t4r")
            nc.vector.tensor_add(w2r[:], v[:, 128:256], ppi[:, 128:256])
            nc.scalar.activation(ntr[:], w2r[:], AFT.Tanh)
            # left tail first: h'_left feeds the next sub-step's T1
            nc.vector.tensor_mul(t4l[:], ntl[:], omz[:, 0:128])
            nc.vector.tensor_add(hnl, t4l[:], zh[:, 0:128])
            # right half on Pool (SBUF-only ops): clears the Vector FIFO so
            # the next sub-step's hTe copy isn't queued behind it; h'_right
            # only feeds T2, which runs after the next rz_even pass anyway
            nc.gpsimd.tensor_mul(t4r[:], ntr[:], omz[:, 128:256])
            nc.gpsimd.tensor_add(hnr, t4r[:], zh[:, 128:256])
            nc.vector.tensor_copy(stage[:, sub * PC:(sub + 1) * PC],
                                  ppi[:, 256:320])

        # Raw pred layout: preds_raw[32j+b, t*64+c] = pred[b, t, 64j+c];
        # the host untangles (j,b) afterwards.  One 2D DMA per body keeps
        # the loop at a single HW-DGE queue (the back-edge drain and the
        # PE's LDWEIGHTS descriptor only support a few sync waits).
        with tc.For_i(0, steps, unroll,
                      hint_engines=(mybir.EngineType.PE,)) as iv:
            stage = None
            if do_tail:
                stage = sb.tile([128, unroll * PC], F32, tag="predstage")
            for s in range(unroll):
                step_body(iv + s, s, stage)
            if not do_tail:
                pass
            elif clobber:  # timing-only build: fixed dst slot
                nc.sync.dma_start(preds[:, 0:unroll * PC], stage[:])
            else:
                nc.sync.dma_start(preds[:, bass.ds(iv * PC, unroll * PC)],
                                  stage[:])


_CACHE = {}


def _get_nc(steps, unroll, clobber=False, out_steps=None, variant="full"):
    key = (steps, unroll, clobber, variant)
    if key in _CACHE:
        return _CACHE[key]
    nc = bacc.Bacc("TRN2", target_bir_lowering=False, debug=False,
                   enable_asserts=False, num_devices=NCORES)
    blob = nc.dram_tensor("blob", [128, BLOBW], FP16,
                          kind="ExternalInput").ap()
    idf = nc.dram_tensor("idf", [128, 128], F32, kind="ExternalInput").ap()
    preds = nc.dram_tensor("preds", [128, (out_steps or steps) * PC], F32,
                           kind="ExternalOutput").ap()
    with tile.TileContext(nc) as tc:
        _emit(tc, nc, blob, idf, preds, steps, unroll, clobber=clobber,
              variant=variant)
    nc.compile()
    _CACHE[key] = nc
    return nc


def _pack(y0_batch, enc_w, enc_b, w_ih, w_hh, bias, bias_n, dec_w, dec_b):
    f = lambda x: np.ascontiguousarray(np.asarray(x, dtype=np.float32))
    y0_batch, enc_w, enc_b = f(y0_batch), f(enc_w), f(enc_b)
    w_ih, w_hh, bias, bias_n = f(w_ih), f(w_hh), f(bias), f(bias_n)
    dec_w, dec_b = f(dec_w), f(dec_b)

    W_r = w_ih[0:H] + w_hh[0:H]
    W_z = w_ih[H:2 * H] + w_hh[H:2 * H]
    W_ni = w_ih[2 * H:3 * H]
    W_nh = w_hh[2 * H:3 * H]

    wcols, bcols = [], []
    for j in range(NG):
        f0, f1 = j * FC, (j + 1) * FC
        p0, p1 = j * PC, (j + 1) * PC
        wcols += [W_r[f0:f1].T, W_z[f0:f1].T, W_nh[f0:f1].T,
                  W_ni[f0:f1].T, dec_w[p0:p1].T]
        bcols += [bias[f0:f1], bias[H + f0:H + f1], bias_n[f0:f1],
                  bias[2 * H + f0:2 * H + f1], dec_b[p0:p1]]

    base = np.zeros((128, BLOBW), np.float32)
    wbig = np.concatenate(wcols, axis=1)            # [1024, 4352]
    base[:, OFF_WS:OFF_ES] = (
        wbig.reshape(KT, 128, NG * GW).transpose(1, 0, 2).reshape(128, -1))
    ebig = np.concatenate(
        [enc_w[j * FC:(j + 1) * FC, :].T for j in range(NG)], axis=1)
    base[:, OFF_ES:OFF_Y0T] = (
        ebig.reshape(2, 128, H).transpose(1, 0, 2).reshape(128, -1))
    base[0, OFF_BR:OFF_EB] = np.concatenate(bcols)
    base[0, OFF_EB:OFF_ID] = np.concatenate(
        [enc_b[j * FC:(j + 1) * FC] for j in range(NG)])
    base[:, OFF_ID:OFF_ONES] = np.eye(128, dtype=np.float32)
    base[0, OFF_ONES:BLOBW] = 1.0

    idf = np.ascontiguousarray(np.eye(128, dtype=np.float32))
    in_maps = []
    for c in range(NCORES):
        bc = base.copy()
        y0t = y0_batch[c * BL:(c + 1) * BL].T       # [256, 32]
        bc[:, OFF_Y0T:OFF_BR] = (
            y0t.reshape(2, 128, BL).transpose(1, 0, 2).reshape(128, -1))
        in_maps.append(dict(blob=bc.astype(np.float16), idf=idf))
    return in_maps


def _pick_unroll(steps):
    for u in (16, 8, 4, 2):
        if steps % u == 0:
            return u
    return 1


def _run(inputs, steps=T, unroll=None, **run_kwargs):
    if unroll is None:
        unroll = _pick_unroll(steps)
    in_maps = _pack(
        inputs["y0_batch"], inputs["enc_w"], inputs["enc_b"], inputs["w_ih"],
        inputs["w_hh"], inputs["bias"], inputs["bias_n"], inputs["dec_w"],
        inputs["dec_b"])
    nc = _get_nc(steps, unroll)
    res = run_bass_kernel_spmd(nc, in_maps, core_ids=list(range(NCORES)),
                               **run_kwargs)
    # preds_raw[32j+b, t*64+c] -> [b, t, 64j+c]
    outs = []
    for r in res.results:
        raw = r["preds"].reshape(NG, BL, steps, PC)
        outs.append(np.ascontiguousarray(raw.transpose(1, 2, 0, 3))
                    .reshape(BL, steps, D))
    return np.concatenate(outs, axis=0), res


def kernel(ts=None, y0_batch=None, enc_w=None, enc_b=None, w_ih=None,
           w_hh=None, bias=None, bias_n=None, dec_w=None, dec_b=None):
    steps = int(np.asarray(ts).shape[0]) if ts is not None else T
    out, _ = _run(dict(y0_batch=y0_batch, enc_w=enc_w, enc_b=enc_b,
                       w_ih=w_ih, w_hh=w_hh, bias=bias, bias_n=bias_n,
                       dec_w=dec_w, dec_b=dec_b), steps=steps)
    return out



# revision 2
# speedup vs baseline: 1.0189x; 1.0189x over previous
"""Trainium2 Bass kernel for nn_DiscreteSequenceModel (GRU rollout) — v2.1.

Math (see reference): h0 = y0 @ enc_w.T + enc_b, then T=512 sequential GRU
steps with input == hidden (ts values are unused by the math; only len(ts)
matters), emitting pred_t = h_t @ dec_w.T + dec_b before each update.

Sharding: data-parallel over batch B=256 across 8 cores (32 rows/core),
weights replicated.  Per core, each step contracts h (K=1024, 8 k-tiles of
128) against a packed weight matrix with 4352 output columns
(r|z|hn|inn|pred per feature-group).  To keep the 128x128 PE array full
with only 32 batch rows, we column-tile the array into 4 groups of 32:
every group holds the same stationary h-tile but streams a different
feature-chunk of the weights, so the array does 4 concurrent 32-wide
matmuls (128 effective rows).

v2 scheduling notes vs v1:
 - PSUM double-buffered by sub-step parity: the next step's matmul passes
   never wait on the previous step's PSUM readers.
 - One sigmoid over [r|z] (Act fixed cost is 352 cycles/inst); 1-z and
   z*h moved to the otherwise-idle Pool (GpSimd) engine (SBUF-only ops).
 - The inn pass is split L/R so tanh(left) starts while the PE still
   streams inn_R|pred; the h' left half lands just in time for the next
   step's transpose, keeping the PE stall-free.

v2.1: the 512-col sigmoid is split into r/z halves (only the r half
gates the critical v -> w2 -> tanh chain; the z half feeds the Pool ops
off-path), and the v = r*hn multiply is split L/R so tanh_L starts a
half-op earlier.
"""

import numpy as np

import concourse.bacc as bacc
import concourse.bass as bass
import concourse.tile as tile
from concourse import mybir
from concourse.bass_utils import run_bass_kernel_spmd

B, T, H, D = 256, 512, 1024, 256
NCORES = 8
BL = B // NCORES            # 32 batch rows per core
NG = 4                      # PE column-tile groups
FC = H // NG                # 256 gate features per group
PC = D // NG                # 64 decoder features per group
GW = 4 * FC + PC            # 1088 packed weight cols per group
KT = H // 128               # 8 k-tiles

# free-dim offsets inside one group's 1088-wide strip: [r|z|hn|inn|pred]
RZ0, RZ1 = 0, 2 * FC             # [r 256 | z 256]
HN0, HN1 = 2 * FC, 3 * FC        # [hn 256]
IA0, IA1 = 3 * FC, 3 * FC + 128  # [inn_L 128]
IB0, IB1 = 3 * FC + 128, GW      # [inn_R 128 | pred 64]

# blob column layout (one [128, BLOBW] fp16 tensor holds every constant)
OFF_WS = 0                       # weights: [p, k*4352 + j*1088 + c]
OFF_ES = OFF_WS + KT * NG * GW   # encoder weights: [p, k*1024 + j*256 + c]
OFF_Y0T = OFF_ES + 2 * H         # y0^T k-tiles: [p, k*32 + b]
OFF_BR = OFF_Y0T + 2 * BL        # row 0: packed gate/pred bias row (4352)
OFF_EB = OFF_BR + NG * GW        # row 0: packed encoder bias row (1024)
OFF_ID = OFF_EB + H              # 128x128 identity (for PE transpose)
OFF_ONES = OFF_ID + 128          # row 0: 32 ones (bias k-tile stationary)
BLOBW = OFF_ONES + BL

F32 = mybir.dt.float32
FP16 = mybir.dt.float16
AFT = mybir.ActivationFunctionType
ALU = mybir.AluOpType


def _emit(tc, nc, blob, idf, preds, steps, unroll, clobber=False,
          variant="full"):
    assert steps % unroll == 0 and unroll % 2 == 0
    import contextlib

    do_mm = variant in ("full", "mmonly")
    do_tail = variant in ("full", "tailonly")

    with contextlib.ExitStack() as ctx:
        const = ctx.enter_context(tc.tile_pool(name="const", bufs=1))
        C = const.tile([128, BLOBW], FP16)
        # h state as separate L/R half-tiles so the elementwise tail can
        # release each half to the PE transposes independently
        HL = [const.tile([128, 128], F32, name="HL0"),
              const.tile([128, 128], F32, name="HL1")]
        HR = [const.tile([128, 128], F32, name="HR0"),
              const.tile([128, 128], F32, name="HR1")]

        IDT = const.tile([128, 128], F32)
        nc.sync.dma_start(C[:], blob[:])
        nc.sync.dma_start(IDT[:], idf[:])

        def ws(k, j, c0, c1):
            o = OFF_WS + k * NG * GW + j * GW
            return C[:, o + c0:o + c1]

        ID = IDT[:]
        ONES = C[0:1, OFF_ONES:OFF_ONES + BL]

        ps = ctx.enter_context(tc.tile_pool(name="ps", bufs=1, space="PSUM"))
        # parity-double-buffered matmul accumulators
        ps_rz = [ps.tile([128, 2 * FC], F32, name="ps_rz0"),
                 ps.tile([128, 2 * FC], F32, name="ps_rz1")]
        ps_hn = [ps.tile([128, FC], F32, name="ps_hn0"),
                 ps.tile([128, FC], F32, name="ps_hn1")]
        ps_pi = [ps.tile([128, FC + PC], F32, name="ps_pi0"),
                 ps.tile([128, FC + PC], F32, name="ps_pi1")]
        # transpose scratch: single-buffered (the SBUF copy consumes it
        # immediately; each tile burns a whole 2KB PSUM bank and only 8 exist)
        ps_t1_s = ps.tile([128, 128], F32, name="ps_t1")
        ps_t2_s = ps.tile([128, 128], F32, name="ps_t2")

        sb = ctx.enter_context(tc.tile_pool(name="sb", bufs=2))

        # ---- encoder: h0 in gate layout [32j+b, f_local] ----
        for j in range(NG):
            for k in range(2):
                nc.tensor.matmul(
                    ps_rz[0][32 * j:32 * j + 32, 0:FC],
                    C[:, OFF_Y0T + k * BL:OFF_Y0T + (k + 1) * BL],
                    C[:, OFF_ES + k * H + j * FC:
                      OFF_ES + k * H + (j + 1) * FC],
                    start=(k == 0), stop=False, skip_group_check=True,
                    tile_position=(0, 32 * j))
            nc.tensor.matmul(
                ps_rz[0][32 * j:32 * j + 32, 0:FC],
                ONES,
                C[0:1, OFF_EB + j * FC:OFF_EB + (j + 1) * FC],
                start=False, stop=True, skip_group_check=True,
                tile_position=(0, 32 * j))
        nc.scalar.copy(HL[0][:], ps_rz[0][:, 0:128])
        nc.vector.tensor_copy(HR[0][:], ps_rz[0][:, 128:256])

        if variant == "mmonly":  # static stationary tiles, no per-step dep
            HTE = const.tile([128, 128], FP16, name="HTE")
            HTO = const.tile([128, 128], FP16, name="HTO")
            nc.scalar.copy(HTE[:], C[:, 0:128])
            nc.vector.tensor_copy(HTO[:], C[:, 0:128])
        if variant == "tailonly":  # PSUM read w/o writer needs an alloc+init
            for p in range(2):
                nc.vector.memset(ps_rz[p][:], 0.0)
                nc.vector.memset(ps_hn[p][:], 0.0)
                nc.vector.memset(ps_pi[p][:], 0.0)

        def step_body(tv, sub, stage):
            parity = sub % 2
            hcl, hcr = HL[parity][:], HR[parity][:]
            hnl, hnr = HL[1 - parity][:], HR[1 - parity][:]
            prz, phn, ppi = ps_rz[parity], ps_hn[parity], ps_pi[parity]
            pt1, pt2 = ps_t1_s, ps_t2_s

            if variant == "mmonly":
                hTe, hTo = HTE, HTO
            else:
                hTe = sb.tile([128, 128], FP16, tag="hTe")
                hTo = sb.tile([128, 128], FP16, tag="hTo")

            def mm(out_ap, k, j, c0, c1, start, stop):
                if k < KT:
                    src = hTe if k % 2 == 0 else hTo
                    m = k // 2
                    lhsT = src[:, m * 32:(m + 1) * 32]
                    rhs = ws(k, j, c0, c1)
                else:  # bias "k-tile": ones x bias row
                    lhsT = ONES
                    rhs = C[0:1, OFF_BR + j * GW + c0:OFF_BR + j * GW + c1]
                nc.tensor.matmul(out_ap, lhsT, rhs,
                                 start=start, stop=stop,
                                 skip_group_check=True,
                                 tile_position=(0, 32 * j))

            # bias "k-tiles" first: constants only, so the PE runs them
            # while the previous sub-step's elementwise tail finishes.
            if do_mm:
                for j in range(NG):
                    mm(prz[32 * j:32 * j + 32, :], KT, j, RZ0, RZ1,
                       True, False)
                for j in range(NG):
                    mm(phn[32 * j:32 * j + 32, :], KT, j, HN0, HN1,
                       True, False)
                # single start=True bias mm covering the whole ps_pi tile:
                # one accumulation group per PSUM bank (the first_mm bank
                # clear makes column-split groups unsafe); the k-tile
                # STREAMS below still split L/R for earlier tanh.
                for j in range(NG):
                    mm(ppi[32 * j:32 * j + 32, 0:320], KT, j, IA0, IB1,
                       True, False)

            # left-half transpose as soon as h_left lands, then the even
            # k-tiles (which live in hTe) while the right half finishes
            if do_tail:
                nc.tensor.transpose(pt1[:], hcl, ID)
                nc.vector.tensor_copy(hTe[:], pt1[:])
            if do_mm:
                for k in range(0, KT, 2):
                    for j in range(NG):
                        mm(prz[32 * j:32 * j + 32, :], k, j, RZ0, RZ1,
                           False, False)
            if do_tail:
                nc.tensor.transpose(pt2[:], hcr, ID)
                nc.vector.tensor_copy(hTo[:], pt2[:])
            if do_mm:
                for k in range(1, KT, 2):
                    for j in range(NG):
                        mm(prz[32 * j:32 * j + 32, :], k, j, RZ0, RZ1,
                           False, k == KT - 1)

            if do_tail:
                rs = sb.tile([128, FC], F32, tag="rs")
                zs = sb.tile([128, FC], F32, tag="zs")
                omz = sb.tile([128, FC], F32, tag="omz")
                zh = sb.tile([128, FC], F32, tag="zh")
                # sigmoid split: only the r half gates the critical chain
                nc.scalar.activation(rs[:], prz[:, 0:FC], AFT.Sigmoid)
                nc.scalar.activation(zs[:], prz[:, FC:2 * FC], AFT.Sigmoid)
                r_, z_ = rs[:], zs[:]
                # Pool (SBUF-only engine): 1-z and z*h off the Vector queue
                nc.gpsimd.tensor_scalar(omz[:], z_, -1.0, 1.0,
                                        ALU.mult, ALU.add)
                nc.gpsimd.tensor_mul(zh[:, 0:128], zs[:, 0:128], hcl)
                nc.gpsimd.tensor_mul(zh[:, 128:256],
                                     zs[:, 128:256], hcr)

            # pass B1: hn — finishes early so v = r*hn hides under B2
            if do_mm:
                for k in range(KT):
                    for j in range(NG):
                        mm(phn[32 * j:32 * j + 32, :], k, j, HN0, HN1,
                           False, k == KT - 1)

            if do_tail:
                # v split: left half feeds tanh_L sooner
                v = sb.tile([128, FC], F32, tag="v")
                nc.vector.tensor_mul(v[:, 0:128], r_[:, 0:128],
                                     phn[:, 0:128])
                nc.vector.tensor_mul(v[:, 128:256], r_[:, 128:256],
                                     phn[:, 128:256])

            # pass B2a: inn_L
            if do_mm:
                for k in range(KT):
                    for j in range(NG):
                        mm(ppi[32 * j:32 * j + 32, 0:128], k, j, IA0, IA1,
                           False, k == KT - 1)

            if do_tail:
                w2l = sb.tile([128, 128], F32, tag="w2l")
                ntl = sb.tile([128, 128], F32, tag="ntl")
                nc.vector.tensor_add(w2l[:], v[:, 0:128], ppi[:, 0:128])
                nc.scalar.activation(ntl[:], w2l[:], AFT.Tanh)

            # pass B2b: inn_R | pred
            if do_mm:
                for k in range(KT):
                    for j in range(NG):
                        mm(ppi[32 * j:32 * j + 32, 128:320], k, j, IB0, IB1,
                           False, k == KT - 1)

            if not do_tail:
                return

            w2r = sb.tile([128, 128], F32, tag="w2r")
            ntr = sb.tile([128, 128], F32, tag="ntr")
            t4l = sb.tile([128, 128], F32, tag="t4l")
            t4r = sb.tile([128, 128], F32, tag="t4r")
            nc.vector.tensor_add(w2r[:], v[:, 128:256], ppi[:, 128:256])
            nc.scalar.activation(ntr[:], w2r[:], AFT.Tanh)
            # left tail first: h'_left feeds the next sub-step's T1
            nc.vector.tensor_mul(t4l[:], ntl[:], omz[:, 0:128])
            nc.vector.tensor_add(hnl, t4l[:], zh[:, 0:128])
            # right half on Pool (SBUF-only ops): clears the Vector FIFO so
            # the next sub-step's hTe copy isn't queued behind it; h'_right
            # only feeds T2, which runs after the next rz_even pass anyway
            nc.gpsimd.tensor_mul(t4r[:], ntr[:], omz[:, 128:256])
            nc.gpsimd.tensor_add(hnr, t4r[:], zh[:, 128:256])
            nc.vector.tensor_copy(stage[:, sub * PC:(sub + 1) * PC],
                                  ppi[:, 256:320])

        # Raw pred layout: preds_raw[32j+b, t*64+c] = pred[b, t, 64j+c];
        # the host untangles (j,b) afterwards.  One 2D DMA per body keeps
        # the loop at a single HW-DGE queue (the back-edge drain and the
        # PE's LDWEIGHTS descriptor only support a few sync waits).
        with tc.For_i(0, steps, unroll,
                      hint_engines=(mybir.EngineType.PE,)) as iv:
            stage = None
            if do_tail:
                stage = sb.tile([128, unroll * PC], F32, tag="predstage")
            for s in range(unroll):
                step_body(iv + s, s, stage)
            if not do_tail:
                pass
            elif clobber:  # timing-only build: fixed dst slot
                nc.sync.dma_start(preds[:, 0:unroll * PC], stage[:])
            else:
                nc.sync.dma_start(preds[:, bass.ds(iv * PC, unroll * PC)],
                                  stage[:])


_CACHE = {}


def _get_nc(steps, unroll, clobber=False, out_steps=None, variant="full"):
    key = (steps, unroll, clobber, variant)
    if key in _CACHE:
        return _CACHE[key]
    nc = bacc.Bacc("TRN2", target_bir_lowering=False, debug=False,
                   enable_asserts=False, num_devices=NCORES)
    blob = nc.dram_tensor("blob", [128, BLOBW], FP16,
                          kind="ExternalInput").ap()
    idf = nc.dram_tensor("idf", [128, 128], F32, kind="ExternalInput").ap()
    preds = nc.dram_tensor("preds", [128, (out_steps or steps) * PC], F32,
                           kind="ExternalOutput").ap()
    with tile.TileContext(nc) as tc:
        _emit(tc, nc, blob, idf, preds, steps, unroll, clobber=clobber,
              variant=variant)
    nc.compile()
    _CACHE[key] = nc
    return nc


def _pack(y0_batch, enc_w, enc_b, w_ih, w_hh, bias, bias_n, dec_w, dec_b):
    f = lambda x: np.ascontiguousarray(np.asarray(x, dtype=np.float32))
    y0_batch, enc_w, enc_b = f(y0_batch), f(enc_w), f(enc_b)
    w_ih, w_hh, bias, bias_n = f(w_ih), f(w_hh), f(bias), f(bias_n)
    dec_w, dec_b = f(dec_w), f(dec_b)

    W_r = w_ih[0:H] + w_hh[0:H]
    W_z = w_ih[H:2 * H] + w_hh[H:2 * H]
    W_ni = w_ih[2 * H:3 * H]
    W_nh = w_hh[2 * H:3 * H]

    wcols, bcols = [], []
    for j in range(NG):
        f0, f1 = j * FC, (j + 1) * FC
        p0, p1 = j * PC, (j + 1) * PC
        wcols += [W_r[f0:f1].T, W_z[f0:f1].T, W_nh[f0:f1].T,
                  W_ni[f0:f1].T, dec_w[p0:p1].T]
        bcols += [bias[f0:f1], bias[H + f0:H + f1], bias_n[f0:f1],
                  bias[2 * H + f0:2 * H + f1], dec_b[p0:p1]]

    base = np.zeros((128, BLOBW), np.float32)
    wbig = np.concatenate(wcols, axis=1)            # [1024, 4352]
    base[:, OFF_WS:OFF_ES] = (
        wbig.reshape(KT, 128, NG * GW).transpose(1, 0, 2).reshape(128, -1))
    ebig = np.concatenate(
        [enc_w[j * FC:(j + 1) * FC, :].T for j in range(NG)], axis=1)
    base[:, OFF_ES:OFF_Y0T] = (
        ebig.reshape(2, 128, H).transpose(1, 0, 2).reshape(128, -1))
    base[0, OFF_BR:OFF_EB] = np.concatenate(bcols)
    base[0, OFF_EB:OFF_ID] = np.concatenate(
        [enc_b[j * FC:(j + 1) * FC] for j in range(NG)])
    base[:, OFF_ID:OFF_ONES] = np.eye(128, dtype=np.float32)
    base[0, OFF_ONES:BLOBW] = 1.0

    idf = np.ascontiguousarray(np.eye(128, dtype=np.float32))
    in_maps = []
    for c in range(NCORES):
        bc = base.copy()
        y0t = y0_batch[c * BL:(c + 1) * BL].T       # [256, 32]
        bc[:, OFF_Y0T:OFF_BR] = (
            y0t.reshape(2, 128, BL).transpose(1, 0, 2).reshape(128, -1))
        in_maps.append(dict(blob=bc.astype(np.float16), idf=idf))
    return in_maps


def _pick_unroll(steps):
    for u in (16, 8, 4, 2):
        if steps % u == 0:
            return u
    return 1


def _run(inputs, steps=T, unroll=None, **run_kwargs):
    if unroll is None:
        unroll = _pick_unroll(steps)
    in_maps = _pack(
        inputs["y0_batch"], inputs["enc_w"], inputs["enc_b"], inputs["w_ih"],
        inputs["w_hh"], inputs["bias"], inputs["bias_n"], inputs["dec_w"],
        inputs["dec_b"])
    nc = _get_nc(steps, unroll)
    res = run_bass_kernel_spmd(nc, in_maps, core_ids=list(range(NCORES)),
                               **run_kwargs)
    # preds_raw[32j+b, t*64+c] -> [b, t, 64j+c]
    outs = []
    for r in res.results:
        raw = r["preds"].reshape(NG, BL, steps, PC)
        outs.append(np.ascontiguousarray(raw.transpose(1, 2, 0, 3))
                    .reshape(BL, steps, D))
    return np.concatenate(outs, axis=0), res


def kernel(ts=None, y0_batch=None, enc_w=None, enc_b=None, w_ih=None,
           w_hh=None, bias=None, bias_n=None, dec_w=None, dec_b=None):
    steps = int(np.asarray(ts).shape[0]) if ts is not None else T
    out, _ = _run(dict(y0_batch=y0_batch, enc_w=enc_w, enc_b=enc_b,
                       w_ih=w_ih, w_hh=w_hh, bias=bias, bias_n=bias_n,
                       dec_w=dec_w, dec_b=dec_b), steps=steps)
    return out

